# revision 47
# baseline (speedup 1.0000x reference)
"""Trainium2 Bass kernel for the CAM (cross-attention module) problem.

Math (per sample b):
    img = w_img @ x1_b          # [256, 4096]
    kv  = w_txt @ x2_b          # [256, 4096]
    attn = softmax(img @ kv^T)  # [256, 256], softmax over last dim
    y = gamma * (attn @ kv) + img
    out_b = w_out @ y           # [512, 4096]

Sharding: data-parallel over batch, 16 samples -> 2 per core x 8 cores,
no collectives.

Precision: projections / logits / attn@kv run in float32r (TRN2's
single-pass fp32 matmul mode) because the softmax is extremely
sensitive to logit error (logits ~ N(0, 64)).  The output conv runs in
bf16 (w_out and y cast to bf16): its error enters the result linearly
(~3e-3 relative), well inside the 2e-2 gate, and bf16 matmuls are much
cheaper than f32r on the PE.

Layout strategy: the spatial contraction (attn logits) needs
spatial-major operands while the residual + output conv need
channel-major ones.  img/kv are computed channel-major from the natural
HBM layout, and the spatial-major copies are made with PE transposes.
The two samples per core are software-pipelined: each sample's tail
(softmax + attn@kv + output conv) is interleaved into the next sample's
projection chunks so the tensor engine never drains.

Schedule details (roofline: ~146us of PE work/core, ~41.5 MB DMA/core):
 - weights/constants stream on the ACT HWDGE queue, x tiles on the SP
   queue (one batched rearranged 3D DMA per tensor per chunk): the
   first matmuls' operands arrive in parallel and DMA-issue slots
   (~650ns each) never pace the pipeline.
 - 8 + 12 dummy ident matmuls, interleaved into the first three chunks,
   keep the PE HAM activity window busy through the DMA-paced head so
   the clock gate opens once (1.2 -> 2.4 GHz) and stays open.
 - x2's 320 channels are NOT host-padded to 384: its three staging
   buffers are persistent tiles whose 64 dead partitions are zeroed
   once from a tiny zeros input, saving 2 MB of HBM reads per core.
 - pipelined-phase output stores alternate the ACT HWDGE and gpsimd
   SWDGE queues (never head-of-line blocking loads on the SP queue);
   the final sample's stores run in the drain, split across the SP and
   ACT queues so store issue never paces the store stream.
 - the last sample's softmax has no later projection work to hide
   behind; ~4 held-back output-conv chunks of the previous sample cover
   its serial DVE/ACT chain so the PE clock never re-throttles.
"""

import numpy as np

# Problem shapes (hardcoded per the harness contract)
B = 16
C1 = 512          # x1 channels (also output channels)
C2 = 320          # x2 channels
C2P = 384         # x2 channels padded to a multiple of 128 (K<128 matmuls
                  # and partition-offset memsets are both broken on HW)
C = 256           # projected channels
HW = 64 * 64      # spatial size
NCORES = 8
SPC = B // NCORES  # samples per core

_BUILD_CACHE = {}


def _nonce_len():
    import inspect
    import zlib
    return 2 + (zlib.crc32(inspect.getsource(_build_nc).encode()) % 997)


def _build_nc(spc=SPC, c1=C1, c2p=C2P, c=C, hw=HW, ch=512):
    """Build the per-core Bass program (same program on all cores)."""
    import concourse.tile as tile
    from concourse import bacc, mybir

    f32 = mybir.dt.float32
    f32r = mybir.dt.float32r
    bf16 = mybir.dt.bfloat16
    P = 128
    K1 = c1 // P           # k-tiles of x1 channels (4)
    K2 = c2p // P          # k-tiles of x2 channels (padded) (3)
    KC = c // P            # k-tiles of projected channels (2)
    MO = c1 // P           # m-tiles of output conv (4)
    NCH = hw // ch         # spatial chunks (8)
    TPC = ch // P          # 128-wide spatial tiles per chunk (4)
    NWARM = 10             # dummy matmuls to lift the HAM clock gate

    # Bacc (not plain Bass): its compile() runs move_matmul_waits_to_ldweights
    # + generate_event_semaphores, without which walrus rejects any Matmult
    # carrying more than one semaphore wait.
    c2 = 320               # true x2 channel count (320 = 2*128 + 64)
    C2T = c2 - 2 * P       # live rows in the last x2 k-tile (64)
    nc = bacc.Bacc("TRN2", target_bir_lowering=False)
    x1 = nc.declare_dram_parameter("x1", [spc, c1, hw], f32r, isOutput=False)
    x2 = nc.declare_dram_parameter("x2", [spc, c2, hw], f32r, isOutput=False)
    zpad = nc.declare_dram_parameter("zpad", [P - C2T, ch], f32r, isOutput=False)
    wiT = nc.declare_dram_parameter("w_imgT", [c1, c], f32r, isOutput=False)
    wtT = nc.declare_dram_parameter("w_txtT", [c2p, c], f32r, isOutput=False)
    woT = nc.declare_dram_parameter("w_outT", [c, c1], bf16, isOutput=False)
    gamma = nc.declare_dram_parameter("gamma", [1], f32, isOutput=False)
    idin = nc.declare_dram_parameter("ident", [P, P], f32r, isOutput=False)
    # The PJRT executable cache fingerprints the HLO without the embedded
    # BIR payload, so two different kernels with identical I/O signatures
    # collide. A source-hash-sized dummy input makes the signature unique.
    nc.declare_dram_parameter("nonce", [1, _nonce_len()], f32, isOutput=False)
    out = nc.declare_dram_parameter("out", [spc, c1, hw], f32, isOutput=True)

    Exp = mybir.ActivationFunctionType.Exp
    X = mybir.AxisListType.X

    with (
        tile.TileContext(nc) as tc,
        tc.tile_pool(name="singles", bufs=1) as singles,
        tc.tile_pool(name="xin", bufs=3) as xin,
        tc.tile_pool(name="tch", bufs=2) as tch,
        tc.tile_pool(name="imgp", bufs=2) as imgp,
        tc.tile_pool(name="kvp", bufs=4) as kvp,
        tc.tile_pool(name="kvbp", bufs=NCH + 2) as kvbp,
        tc.tile_pool(name="ybp", bufs=3) as ybp,
        tc.tile_pool(name="attnsb", bufs=2) as attnsb,
        tc.tile_pool(name="smalls", bufs=4) as smalls,
        tc.tile_pool(name="ostage", bufs=8) as ostage,
        tc.tile_pool(name="psA", bufs=4, space="PSUM") as psA,
        tc.tile_pool(name="psB", bufs=2, space="PSUM") as psB,
        tc.tile_pool(name="psAttn", bufs=2, space="PSUM") as psAttn,
    ):
        # ---- constants: ident first on the SP queue (tiny - unblocks the
        # PE warm-up immediately); the batched weight loads stream on the
        # ACT HWDGE queue in parallel with the x loads on the SP queue.
        wiT_sb = singles.tile([P, K1, c], f32r)
        wtT_sb = singles.tile([P, K2, c], f32r)
        woT_sb = singles.tile([P, KC, c1], bf16)
        ident = singles.tile([P, P], f32r)
        ident_bf = singles.tile([P, P], bf16)
        gamma_sb = singles.tile([P, 1], f32)

        nc.sync.dma_start(out=ident, in_=idin[:])
        # one 3D DMA per weight tensor: SBUF[p, k, c] <- W[k*128 + p, c]
        nc.scalar.dma_start(out=wiT_sb[:, :, :],
                            in_=wiT[:, :].rearrange("(k p) c -> p k c", p=P))
        nc.scalar.dma_start(out=wtT_sb[:, :, :],
                            in_=wtT[:, :].rearrange("(k p) c -> p k c", p=P))
        nc.scalar.dma_start(out=woT_sb[:, :, :],
                            in_=woT[:, :].rearrange("(k p) c -> p k c", p=P))
        nc.scalar.dma_start(out=gamma_sb, in_=gamma[:].to_broadcast((P, 1)))
        nc.vector.tensor_copy(out=ident_bf, in_=ident)

        # x2 staging: 3 persistent buffers rotated manually (a tile pool
        # would treat the one-time pad zeroing below as a cross-generation
        # alias).  The dead partitions [C2T:, k=2] are zeroed once from the
        # zpad zeros input; all later DMAs only touch live regions.
        x2cs = [singles.tile([P, K2, ch], f32r, name=f"x2cbuf{i}")
                for i in range(3)]
        nc.sync.dma_start(out=x2cs[0][C2T:, K2 - 1, :], in_=zpad[:, :])
        nc.scalar.dma_start(out=x2cs[1][C2T:, K2 - 1, :], in_=zpad[:, :])
        nc.scalar.dma_start(out=x2cs[2][C2T:, K2 - 1, :], in_=zpad[:, :])

        # ---- PE clock warm-up: dummy N=128 matmuls on ident.  They only
        # depend on ident (first DMA) and write psA rotation slots nothing
        # reads.  A burst here + trickles into the first chunks (emitted in
        # passA_chunk below) keeps the HAM activity window busy through the
        # DMA-paced head so the clock gate opens once and stays open.
        def warmup(n):
            for _ in range(n):
                wps = psA.tile([P, ch], f32, tag="a", name="ps_warm")
                nc.tensor.matmul(wps[:, 0:P], lhsT=ident, rhs=ident,
                                 start=True, stop=True)

        warmup(8)

        x2c_allocs = [0]  # x2c pool allocation counter (pad-init trick)

        # ---- per-sample emission helpers -------------------------------
        def passA_chunk(st, s, cc, first=False):
            cs = cc * ch
            x1c = xin.tile([P, K1, ch], f32r, tag="x1c", name="x1c")
            x2c = x2cs[x2c_allocs[0] % 3]
            x2c_allocs[0] += 1
            if first:
                # k-granular, x1/x2 interleaved: minimizes the latency to
                # the first img and kv matmul chains of the whole kernel
                for k in range(K1):
                    nc.sync.dma_start(out=x1c[:, k, :],
                                      in_=x1[s, k * P:(k + 1) * P, cs:cs + ch])
                    if k < 2:
                        nc.sync.dma_start(out=x2c[:, k, :],
                                          in_=x2[s, k * P:(k + 1) * P, cs:cs + ch])
                    elif k == 2:
                        nc.sync.dma_start(
                            out=x2c[:C2T, 2, :],
                            in_=x2[s, 2 * P:2 * P + C2T, cs:cs + ch])
            else:
                # batched 3D DMAs (HWDGE issue slots are ~650ns each -
                # 7 per chunk would pace the whole pipeline)
                nc.sync.dma_start(
                    out=x1c[:, :, :],
                    in_=x1[s, :, cs:cs + ch].rearrange("(k p) c -> p k c", p=P))
                nc.sync.dma_start(
                    out=x2c[:, :2, :],
                    in_=x2[s, :2 * P, cs:cs + ch].rearrange("(k p) c -> p k c",
                                                            p=P))
                nc.sync.dma_start(out=x2c[:C2T, 2, :],
                                  in_=x2[s, 2 * P:2 * P + C2T, cs:cs + ch])
            for m in range(KC):
                ps = psA.tile([P, ch], f32, tag="a", name="ps_img")
                for k in range(K1):
                    nc.tensor.matmul(ps, lhsT=wiT_sb[:, k, m * P:(m + 1) * P],
                                     rhs=x1c[:, k, :],
                                     start=(k == 0), stop=(k == K1 - 1))
                nc.vector.tensor_copy(out=st["img"][:, m, cs:cs + ch], in_=ps)
            kvt = kvp.tile([P, KC, ch], f32r, tag="kv", name="kvt")
            kvb = kvbp.tile([P, KC, ch], bf16, tag="kvb", name="kvb")
            st["kvch"][cc] = kvt
            st["kvb"][cc] = kvb
            for m in range(KC):
                ps = psA.tile([P, ch], f32, tag="a", name="ps_kv")
                for k in range(K2):
                    nc.tensor.matmul(ps, lhsT=wtT_sb[:, k, m * P:(m + 1) * P],
                                     rhs=x2c[:, k, :],
                                     start=(k == 0), stop=(k == K2 - 1))
                nc.vector.tensor_copy(out=kvt[:, m, :], in_=ps)
            # bf16 shadow of kv for the attn@kv bmm: one SBUF->SBUF copy
            # (DVE 2x perf mode), so the f32r copy only needs to live until
            # the next chunk's transposes
            nc.vector.tensor_copy(out=kvb[:, :, :], in_=kvt[:, :, :])

        def transposes(st, s, pc):
            # spatial-major orientations via PE transpose of img / kv chunks
            pcs = pc * ch
            imgT_c = tch.tile([P, TPC, c], f32r, tag="imgT", name="imgT_c")
            for t in range(TPC):
                ps = psB.tile([P, c], f32r, tag="b", name="ps_imgT")
                for i in range(KC):
                    nc.tensor.transpose(
                        ps[:, i * P:(i + 1) * P],
                        st["img"][:, i, pcs + t * P:pcs + (t + 1) * P], ident)
                nc.scalar.copy(out=imgT_c[:, t, :], in_=ps)
            txtT_c = tch.tile([P, TPC, c], f32r, tag="txtT", name="txtT_c")
            for t in range(TPC):
                ps = psB.tile([P, c], f32r, tag="b", name="ps_txtT")
                for i in range(KC):
                    nc.tensor.transpose(
                        ps[:, i * P:(i + 1) * P],
                        st["kvch"][pc][:, i, t * P:(t + 1) * P], ident)
                nc.scalar.copy(out=txtT_c[:, t, :], in_=ps)
            st["kvch"][pc] = None
            st["imgT"][pc] = imgT_c
            st["txtT"][pc] = txtT_c

        def attn_chunk(st, s, pc):
            if st["attn_ps"] is None:
                st["attn_ps"] = [
                    psAttn.tile([P, c], f32, tag="attn", name=f"attn{s}_{m}")
                    for m in range(KC)
                ]
            for m in range(KC):
                for t in range(TPC):
                    nc.tensor.matmul(
                        st["attn_ps"][m],
                        lhsT=st["imgT"][pc][:, t, m * P:(m + 1) * P],
                        rhs=st["txtT"][pc][:, t, :],
                        start=(pc == 0 and t == 0),
                        stop=(pc == NCH - 1 and t == TPC - 1))
            st["imgT"][pc] = st["txtT"][pc] = None

        def softmax(st, s, cover=()):
            # softmax over the free (d) axis, gamma folded in; transpose to
            # attnT [d, c] for the attn@kv contraction.  `cover` closures are
            # emitted between the DVE/ACT stats and the PE transposes so the
            # tensor engine has work while the serial softmax chain runs.
            attnT_sb = attnsb.tile([P, KC, c], bf16, tag="attnT", name="attnT")
            st["attnT"] = attnT_sb
            exps = []
            for m in range(KC):
                nmax = smalls.tile([P, 1], f32, tag="nmax", name="nmax")
                nc.vector.reduce_max(out=nmax, in_=st["attn_ps"][m], axis=X,
                                     negate=True)
                exp_sb = smalls.tile([P, c], f32r, tag="exp", name="exp_sb")
                rsum = smalls.tile([P, 1], f32, tag="rsum", name="rsum")
                nc.scalar.activation(out=exp_sb, in_=st["attn_ps"][m], func=Exp,
                                     bias=nmax, scale=1.0, accum_out=rsum)
                rg = smalls.tile([P, 1], f32, tag="rg", name="rg")
                nc.vector.reciprocal(out=rg, in_=rsum)
                nc.vector.tensor_mul(out=rg, in0=rg, in1=gamma_sb)
                # attn weights (gamma folded) cast to bf16 for attn@kv
                exp_bf = smalls.tile([P, c], bf16, tag="expb", name="exp_bf")
                nc.vector.tensor_scalar_mul(out=exp_bf, in0=exp_sb, scalar1=rg)
                exps.append(exp_bf)
            for fn in cover:
                fn()
            for m in range(KC):
                for j in range(KC):
                    pst = psB.tile([P, P], bf16, tag="b", name="ps_tr")
                    nc.tensor.transpose(pst, exps[m][:, j * P:(j + 1) * P], ident_bf)
                    nc.vector.tensor_copy(out=attnT_sb[:, j, m * P:(m + 1) * P],
                                          in_=pst)

        def ph4_chunk(st, s, cc):
            # y = gamma*attn@kv + img, cast to bf16 for the output conv
            # (GPSIMD cannot read PSUM, so the residual add stays on DVE)
            cs = cc * ch
            yb = ybp.tile([P, KC, ch], bf16, tag="yb", name="yb")
            st["ybch"][cc] = yb
            for m in range(KC):
                ps = psA.tile([P, ch], f32, tag="a", name="ps_ai")
                for j in range(KC):
                    nc.tensor.matmul(ps, lhsT=st["attnT"][:, j, m * P:(m + 1) * P],
                                     rhs=st["kvb"][cc][:, j, :],
                                     start=(j == 0), stop=(j == KC - 1))
                nc.vector.tensor_add(out=yb[:, m, :], in0=ps,
                                     in1=st["img"][:, m, cs:cs + ch])
            st["kvb"][cc] = None

        def ph5_chunk(st, s, cc):
            cs = cc * ch
            hh = ch // 2
            yb = st["ybch"][cc]
            for m2 in range(MO):
                ps = psA.tile([P, ch], f32, tag="a", name="ps_out")
                for j in range(KC):
                    nc.tensor.matmul(ps, lhsT=woT_sb[:, j, m2 * P:(m2 + 1) * P],
                                     rhs=yb[:, j, :],
                                     start=(j == 0), stop=(j == KC - 1))
                # evacuation split DVE/ACT: halves the psum-bank hold time
                ot = ostage.tile([P, ch], f32, tag="ot", name="ot")
                nc.vector.tensor_copy(out=ot[:, :hh], in_=ps[:, :hh])
                nc.scalar.copy(out=ot[:, hh:], in_=ps[:, hh:])
                # pipelined-phase stores alternate the ACT HWDGE queue and
                # the gpsimd SWDGE queue (so they never head-of-line block
                # the loads on the SP queue); the final sample's stores run
                # in the drain where the SP queue is idle and fastest.
                if s < spc - 1:
                    deng = nc.scalar if m2 % 2 == 0 else nc.gpsimd
                else:
                    # drain stores split across both HWDGE queues so the
                    # ~620ns per-issue cost never paces the store stream
                    deng = nc.sync if m2 % 2 == 0 else nc.scalar
                deng.dma_start(out=out[s, m2 * P:(m2 + 1) * P, cs:cs + ch],
                               in_=ot)
            st["ybch"][cc] = None

        # ---- pipelined schedule: sample s-1's tail (last transposes, attn,
        # softmax, phases 4/5) is interleaved into sample s's pass-A chunks
        # so the PE never drains at sample boundaries.
        tails = []
        for s in range(spc):
            st = {"img": None, "kvch": [None] * NCH, "kvb": [None] * NCH,
                  "attn_ps": None, "attnT": None, "imgT": [None] * NCH,
                  "txtT": [None] * NCH, "ybch": [None] * NCH}
            st["img"] = imgp.tile([P, KC, hw], f32r, tag="img", name=f"img{s}")
            for cc in range(NCH):
                passA_chunk(st, s, cc, first=(s == 0 and cc == 0))
                if s == 0 and cc < 3:
                    warmup(4)
                if cc >= 1:
                    transposes(st, s, cc - 1)
                if cc >= 2:
                    attn_chunk(st, s, cc - 2)
                # 4 pops at cc=0 gets the previous sample's softmax emitted
                # before this sample's attn chains need the psAttn banks;
                # popping only 16 of 20 leaves ~4.5us of PE work to cover
                # the final softmax's serial DVE/ACT chain
                npop = (4, 3, 3, 2, 2, 1, 1, 0)[min(cc, 7)]
                for _ in range(npop):
                    if tails:
                        tails.pop(0)()
            if s == spc - 1:
                # the final sample's softmax has no later pass-A to hide
                # behind; cover it with whatever of the previous sample's
                # tail is still pending (its last output-conv chunks).
                leftovers = tails[:]
                tails.clear()
                tails.extend([
                    (lambda st=st, s=s: transposes(st, s, NCH - 1)),
                    (lambda st=st, s=s: attn_chunk(st, s, NCH - 2)),
                    (lambda st=st, s=s: attn_chunk(st, s, NCH - 1)),
                    (lambda st=st, s=s, cov=tuple(leftovers):
                        softmax(st, s, cover=cov)),
                ])
            else:
                tails.extend([
                    (lambda st=st, s=s: transposes(st, s, NCH - 1)),
                    (lambda st=st, s=s: attn_chunk(st, s, NCH - 2)),
                    (lambda st=st, s=s: attn_chunk(st, s, NCH - 1)),
                    (lambda st=st, s=s: softmax(st, s)),
                ])
            # interleave attn@kv and output-conv chunks with a 1-chunk lag
            # so the DVE residual add of chunk c hides under the PE work of
            # chunk c+1
            tails.append(lambda st=st, s=s: ph4_chunk(st, s, 0))
            for cc in range(1, NCH):
                tails.append(lambda st=st, s=s, cc=cc: ph4_chunk(st, s, cc))
                tails.append(lambda st=st, s=s, cc=cc: ph5_chunk(st, s, cc - 1))
            tails.append(lambda st=st, s=s: ph5_chunk(st, s, NCH - 1))
        while tails:
            tails.pop(0)()

    nc.compile()
    return nc


def _get_nc():
    key = "full"
    if key not in _BUILD_CACHE:
        _BUILD_CACHE[key] = _build_nc()
    return _BUILD_CACHE[key]


LAST_RESULTS = None  # BassKernelResults of the most recent kernel() call


def kernel(x1, x2, w_img, w_txt, w_out, gamma):
    import os
    import ml_dtypes
    from concourse.bass_utils import run_bass_kernel_spmd


    x1 = np.ascontiguousarray(np.asarray(x1, dtype=np.float32)).reshape(B, C1, HW)
    x2 = np.ascontiguousarray(np.asarray(x2, dtype=np.float32)).reshape(B, C2, HW)
    w_img = np.asarray(w_img, dtype=np.float32)
    w_txt = np.asarray(w_txt, dtype=np.float32)
    w_out = np.asarray(w_out, dtype=np.float32)
    gamma = np.ascontiguousarray(np.asarray(gamma, dtype=np.float32)).reshape(1)

    w_imgT = np.ascontiguousarray(w_img.T)              # [512, 256] f32
    w_txtT = np.zeros((C2P, C), dtype=np.float32)       # [384, 256] f32, zero-pad
    w_txtT[:C2, :] = w_txt.T
    w_outT = np.ascontiguousarray(w_out.T.astype(ml_dtypes.bfloat16))  # [256, 512]

    nc = _get_nc()
    ident = np.eye(128, dtype=np.float32)
    zpad = np.zeros((64, 512), dtype=np.float32)
    in_maps = []
    for core in range(NCORES):
        s0 = core * SPC
        in_maps.append({
            "x1": np.ascontiguousarray(x1[s0:s0 + SPC]),
            "x2": np.ascontiguousarray(x2[s0:s0 + SPC]),
            "w_imgT": w_imgT,
            "w_txtT": w_txtT,
            "w_outT": w_outT,
            "gamma": gamma,
            "ident": ident,
            "zpad": zpad,
            "nonce": np.zeros((1, _nonce_len()), dtype=np.float32),
        })

    kwargs = {}
    if os.environ.get("KERNEL_TRACE"):
        kwargs["trace"] = True
        if os.environ.get("KERNEL_TRACE_DIR"):
            kwargs["tmpdir"] = os.environ["KERNEL_TRACE_DIR"]
    res = run_bass_kernel_spmd(nc, in_maps, core_ids=list(range(NCORES)), **kwargs)
    global LAST_RESULTS
    LAST_RESULTS = res
    outs = [res.results[c]["out"] for c in range(NCORES)]
    full = np.concatenate(outs, axis=0).reshape(B, C1, 64, 64)
    return full


if __name__ == "__main__":
    rng = np.random.default_rng(0)
    inputs = {
        "x1": rng.standard_normal((B, C1, 64, 64), dtype=np.float32),
        "x2": rng.standard_normal((B, C2, 64, 64), dtype=np.float32),
        "w_img": rng.standard_normal((C, C1), dtype=np.float32) / np.sqrt(C1),
        "w_txt": rng.standard_normal((C, C2), dtype=np.float32) / np.sqrt(C2),
        "w_out": rng.standard_normal((C1, C), dtype=np.float32) / np.sqrt(C),
        "gamma": rng.standard_normal(1).astype(np.float32),
    }
    out = kernel(**inputs)
    print(out.shape, out.dtype)



# revision 48
# speedup vs baseline: 1.0028x; 1.0028x over previous
"""Trainium2 Bass kernel for the CAM (cross-attention module) problem.

Math (per sample b):
    img = w_img @ x1_b          # [256, 4096]
    kv  = w_txt @ x2_b          # [256, 4096]
    attn = softmax(img @ kv^T)  # [256, 256], softmax over last dim
    y = gamma * (attn @ kv) + img
    out_b = w_out @ y           # [512, 4096]

Sharding: data-parallel over batch, 16 samples -> 2 per core x 8 cores,
no collectives.

Precision: projections / logits / attn@kv run in float32r (TRN2's
single-pass fp32 matmul mode) because the softmax is extremely
sensitive to logit error (logits ~ N(0, 64)).  The output conv runs in
bf16 (w_out and y cast to bf16): its error enters the result linearly
(~3e-3 relative), well inside the 2e-2 gate, and bf16 matmuls are much
cheaper than f32r on the PE.

Layout strategy: the spatial contraction (attn logits) needs
spatial-major operands while the residual + output conv need
channel-major ones.  img/kv are computed channel-major from the natural
HBM layout, and the spatial-major copies are made with PE transposes.
The two samples per core are software-pipelined: each sample's tail
(softmax + attn@kv + output conv) is interleaved into the next sample's
projection chunks so the tensor engine never drains.

Schedule details (roofline: ~146us of PE work/core, ~41.5 MB DMA/core):
 - weights/constants stream on the ACT HWDGE queue, x tiles on the SP
   queue (one batched rearranged 3D DMA per tensor per chunk): the
   first matmuls' operands arrive in parallel and DMA-issue slots
   (~650ns each) never pace the pipeline.
 - 8 + 12 dummy ident matmuls, interleaved into the first three chunks,
   keep the PE HAM activity window busy through the DMA-paced head so
   the clock gate opens once (1.2 -> 2.4 GHz) and stays open.
 - x2's 320 channels are NOT host-padded to 384: its three staging
   buffers are persistent tiles whose 64 dead partitions are zeroed
   once from a tiny zeros input, saving 2 MB of HBM reads per core.
 - pipelined-phase output stores alternate the ACT HWDGE and gpsimd
   SWDGE queues (never head-of-line blocking loads on the SP queue);
   the final sample's stores run in the drain, split across the SP and
   ACT queues so store issue never paces the store stream.
 - the last sample's softmax has no later projection work to hide
   behind; ~4 held-back output-conv chunks of the previous sample cover
   its serial DVE/ACT chain so the PE clock never re-throttles.
"""

import numpy as np

# Problem shapes (hardcoded per the harness contract)
B = 16
C1 = 512          # x1 channels (also output channels)
C2 = 320          # x2 channels
C2P = 384         # x2 channels padded to a multiple of 128 (K<128 matmuls
                  # and partition-offset memsets are both broken on HW)
C = 256           # projected channels
HW = 64 * 64      # spatial size
NCORES = 8
SPC = B // NCORES  # samples per core

_BUILD_CACHE = {}


def _nonce_len():
    import inspect
    import zlib
    return 2 + (zlib.crc32(inspect.getsource(_build_nc).encode()) % 997)


def _build_nc(spc=SPC, c1=C1, c2p=C2P, c=C, hw=HW, ch=512):
    """Build the per-core Bass program (same program on all cores)."""
    import concourse.tile as tile
    from concourse import bacc, mybir

    f32 = mybir.dt.float32
    f32r = mybir.dt.float32r
    bf16 = mybir.dt.bfloat16
    P = 128
    K1 = c1 // P           # k-tiles of x1 channels (4)
    K2 = c2p // P          # k-tiles of x2 channels (padded) (3)
    KC = c // P            # k-tiles of projected channels (2)
    MO = c1 // P           # m-tiles of output conv (4)
    NCH = hw // ch         # spatial chunks (8)
    TPC = ch // P          # 128-wide spatial tiles per chunk (4)
    NWARM = 10             # dummy matmuls to lift the HAM clock gate

    # Bacc (not plain Bass): its compile() runs move_matmul_waits_to_ldweights
    # + generate_event_semaphores, without which walrus rejects any Matmult
    # carrying more than one semaphore wait.
    c2 = 320               # true x2 channel count (320 = 2*128 + 64)
    C2T = c2 - 2 * P       # live rows in the last x2 k-tile (64)
    nc = bacc.Bacc("TRN2", target_bir_lowering=False)
    x1 = nc.declare_dram_parameter("x1", [spc, c1, hw], f32r, isOutput=False)
    x2 = nc.declare_dram_parameter("x2", [spc, c2, hw], f32r, isOutput=False)
    zpad = nc.declare_dram_parameter("zpad", [P - C2T, ch], f32r, isOutput=False)
    wiT = nc.declare_dram_parameter("w_imgT", [c1, c], f32r, isOutput=False)
    wtT = nc.declare_dram_parameter("w_txtT", [c2p, c], f32r, isOutput=False)
    woT = nc.declare_dram_parameter("w_outT", [c, c1], bf16, isOutput=False)
    gamma = nc.declare_dram_parameter("gamma", [1], f32, isOutput=False)
    idin = nc.declare_dram_parameter("ident", [P, P], f32r, isOutput=False)
    # The PJRT executable cache fingerprints the HLO without the embedded
    # BIR payload, so two different kernels with identical I/O signatures
    # collide. A source-hash-sized dummy input makes the signature unique.
    nc.declare_dram_parameter("nonce", [1, _nonce_len()], f32, isOutput=False)
    out = nc.declare_dram_parameter("out", [spc, c1, hw], f32, isOutput=True)

    Exp = mybir.ActivationFunctionType.Exp
    X = mybir.AxisListType.X

    with (
        tile.TileContext(nc) as tc,
        tc.tile_pool(name="singles", bufs=1) as singles,
        tc.tile_pool(name="xin", bufs=3) as xin,
        tc.tile_pool(name="tch", bufs=2) as tch,
        tc.tile_pool(name="imgp", bufs=2) as imgp,
        tc.tile_pool(name="kvp", bufs=NCH + 2) as kvp,
        tc.tile_pool(name="ybp", bufs=3) as ybp,
        tc.tile_pool(name="attnsb", bufs=2) as attnsb,
        tc.tile_pool(name="smalls", bufs=4) as smalls,
        tc.tile_pool(name="ostage", bufs=8) as ostage,
        tc.tile_pool(name="psA", bufs=4, space="PSUM") as psA,
        tc.tile_pool(name="psB", bufs=2, space="PSUM") as psB,
        tc.tile_pool(name="psAttn", bufs=2, space="PSUM") as psAttn,
    ):
        # ---- constants: ident first on the SP queue (tiny - unblocks the
        # PE warm-up immediately); the batched weight loads stream on the
        # ACT HWDGE queue in parallel with the x loads on the SP queue.
        wiT_sb = singles.tile([P, K1, c], f32r)
        wtT_sb = singles.tile([P, K2, c], f32r)
        woT_sb = singles.tile([P, KC, c1], bf16)
        ident = singles.tile([P, P], f32r)
        gamma_sb = singles.tile([P, 1], f32)

        nc.sync.dma_start(out=ident, in_=idin[:])
        # one 3D DMA per weight tensor: SBUF[p, k, c] <- W[k*128 + p, c]
        nc.scalar.dma_start(out=wiT_sb[:, :, :],
                            in_=wiT[:, :].rearrange("(k p) c -> p k c", p=P))
        nc.scalar.dma_start(out=wtT_sb[:, :, :],
                            in_=wtT[:, :].rearrange("(k p) c -> p k c", p=P))
        nc.scalar.dma_start(out=woT_sb[:, :, :],
                            in_=woT[:, :].rearrange("(k p) c -> p k c", p=P))
        nc.scalar.dma_start(out=gamma_sb, in_=gamma[:].to_broadcast((P, 1)))

        # x2 staging: 3 persistent buffers rotated manually (a tile pool
        # would treat the one-time pad zeroing below as a cross-generation
        # alias).  The dead partitions [C2T:, k=2] are zeroed once from the
        # zpad zeros input; all later DMAs only touch live regions.
        x2cs = [singles.tile([P, K2, ch], f32r, name=f"x2cbuf{i}")
                for i in range(3)]
        nc.sync.dma_start(out=x2cs[0][C2T:, K2 - 1, :], in_=zpad[:, :])
        nc.scalar.dma_start(out=x2cs[1][C2T:, K2 - 1, :], in_=zpad[:, :])
        nc.scalar.dma_start(out=x2cs[2][C2T:, K2 - 1, :], in_=zpad[:, :])

        # ---- PE clock warm-up: dummy N=128 matmuls on ident.  They only
        # depend on ident (first DMA) and write psA rotation slots nothing
        # reads.  A burst here + trickles into the first chunks (emitted in
        # passA_chunk below) keeps the HAM activity window busy through the
        # DMA-paced head so the clock gate opens once and stays open.
        def warmup(n):
            for _ in range(n):
                wps = psA.tile([P, ch], f32, tag="a", name="ps_warm")
                nc.tensor.matmul(wps[:, 0:P], lhsT=ident, rhs=ident,
                                 start=True, stop=True)

        warmup(8)

        x2c_allocs = [0]  # x2c pool allocation counter (pad-init trick)

        # ---- per-sample emission helpers -------------------------------
        def passA_chunk(st, s, cc, first=False):
            cs = cc * ch
            x1c = xin.tile([P, K1, ch], f32r, tag="x1c", name="x1c")
            x2c = x2cs[x2c_allocs[0] % 3]
            x2c_allocs[0] += 1
            if first:
                # k-granular, x1/x2 interleaved: minimizes the latency to
                # the first img and kv matmul chains of the whole kernel
                for k in range(K1):
                    nc.sync.dma_start(out=x1c[:, k, :],
                                      in_=x1[s, k * P:(k + 1) * P, cs:cs + ch])
                    if k < 2:
                        nc.sync.dma_start(out=x2c[:, k, :],
                                          in_=x2[s, k * P:(k + 1) * P, cs:cs + ch])
                    elif k == 2:
                        nc.sync.dma_start(
                            out=x2c[:C2T, 2, :],
                            in_=x2[s, 2 * P:2 * P + C2T, cs:cs + ch])
            else:
                # batched 3D DMAs (HWDGE issue slots are ~650ns each -
                # 7 per chunk would pace the whole pipeline)
                nc.sync.dma_start(
                    out=x1c[:, :, :],
                    in_=x1[s, :, cs:cs + ch].rearrange("(k p) c -> p k c", p=P))
                nc.sync.dma_start(
                    out=x2c[:, :2, :],
                    in_=x2[s, :2 * P, cs:cs + ch].rearrange("(k p) c -> p k c",
                                                            p=P))
                nc.sync.dma_start(out=x2c[:C2T, 2, :],
                                  in_=x2[s, 2 * P:2 * P + C2T, cs:cs + ch])
            for m in range(KC):
                ps = psA.tile([P, ch], f32, tag="a", name="ps_img")
                for k in range(K1):
                    nc.tensor.matmul(ps, lhsT=wiT_sb[:, k, m * P:(m + 1) * P],
                                     rhs=x1c[:, k, :],
                                     start=(k == 0), stop=(k == K1 - 1))
                nc.vector.tensor_copy(out=st["img"][:, m, cs:cs + ch], in_=ps)
            kvt = kvp.tile([P, KC, ch], f32r, tag="kv", name="kvt")
            st["kvch"][cc] = kvt
            for m in range(KC):
                ps = psA.tile([P, ch], f32, tag="a", name="ps_kv")
                for k in range(K2):
                    nc.tensor.matmul(ps, lhsT=wtT_sb[:, k, m * P:(m + 1) * P],
                                     rhs=x2c[:, k, :],
                                     start=(k == 0), stop=(k == K2 - 1))
                nc.vector.tensor_copy(out=kvt[:, m, :], in_=ps)

        def transposes(st, s, pc):
            # spatial-major orientations via PE transpose of img / kv chunks
            pcs = pc * ch
            imgT_c = tch.tile([P, TPC, c], f32r, tag="imgT", name="imgT_c")
            for t in range(TPC):
                ps = psB.tile([P, c], f32r, tag="b", name="ps_imgT")
                for i in range(KC):
                    nc.tensor.transpose(
                        ps[:, i * P:(i + 1) * P],
                        st["img"][:, i, pcs + t * P:pcs + (t + 1) * P], ident)
                nc.scalar.copy(out=imgT_c[:, t, :], in_=ps)
            txtT_c = tch.tile([P, TPC, c], f32r, tag="txtT", name="txtT_c")
            for t in range(TPC):
                ps = psB.tile([P, c], f32r, tag="b", name="ps_txtT")
                for i in range(KC):
                    nc.tensor.transpose(
                        ps[:, i * P:(i + 1) * P],
                        st["kvch"][pc][:, i, t * P:(t + 1) * P], ident)
                nc.scalar.copy(out=txtT_c[:, t, :], in_=ps)
            st["imgT"][pc] = imgT_c
            st["txtT"][pc] = txtT_c

        def attn_chunk(st, s, pc):
            if st["attn_ps"] is None:
                st["attn_ps"] = [
                    psAttn.tile([P, c], f32, tag="attn", name=f"attn{s}_{m}")
                    for m in range(KC)
                ]
            for m in range(KC):
                for t in range(TPC):
                    nc.tensor.matmul(
                        st["attn_ps"][m],
                        lhsT=st["imgT"][pc][:, t, m * P:(m + 1) * P],
                        rhs=st["txtT"][pc][:, t, :],
                        start=(pc == 0 and t == 0),
                        stop=(pc == NCH - 1 and t == TPC - 1))
            st["imgT"][pc] = st["txtT"][pc] = None

        def softmax(st, s, cover=()):
            # softmax over the free (d) axis, gamma folded in; transpose to
            # attnT [d, c] for the attn@kv contraction.  `cover` closures are
            # emitted between the DVE/ACT stats and the PE transposes so the
            # tensor engine has work while the serial softmax chain runs.
            attnT_sb = attnsb.tile([P, KC, c], f32r, tag="attnT", name="attnT")
            st["attnT"] = attnT_sb
            exps = []
            for m in range(KC):
                nmax = smalls.tile([P, 1], f32, tag="nmax", name="nmax")
                nc.vector.reduce_max(out=nmax, in_=st["attn_ps"][m], axis=X,
                                     negate=True)
                exp_sb = smalls.tile([P, c], f32r, tag="exp", name="exp_sb")
                rsum = smalls.tile([P, 1], f32, tag="rsum", name="rsum")
                nc.scalar.activation(out=exp_sb, in_=st["attn_ps"][m], func=Exp,
                                     bias=nmax, scale=1.0, accum_out=rsum)
                rg = smalls.tile([P, 1], f32, tag="rg", name="rg")
                nc.vector.reciprocal(out=rg, in_=rsum)
                nc.vector.tensor_mul(out=rg, in0=rg, in1=gamma_sb)
                nc.vector.tensor_scalar_mul(out=exp_sb, in0=exp_sb, scalar1=rg)
                exps.append(exp_sb)
            for fn in cover:
                fn()
            for m in range(KC):
                for j in range(KC):
                    pst = psB.tile([P, P], f32r, tag="b", name="ps_tr")
                    nc.tensor.transpose(pst, exps[m][:, j * P:(j + 1) * P], ident)
                    nc.vector.tensor_copy(out=attnT_sb[:, j, m * P:(m + 1) * P],
                                          in_=pst)

        def ph4_chunk(st, s, cc):
            # y = gamma*attn@kv + img, cast to bf16 for the output conv
            # (GPSIMD cannot read PSUM, so the residual add stays on DVE)
            cs = cc * ch
            yb = ybp.tile([P, KC, ch], bf16, tag="yb", name="yb")
            st["ybch"][cc] = yb
            for m in range(KC):
                ps = psA.tile([P, ch], f32, tag="a", name="ps_ai")
                for j in range(KC):
                    nc.tensor.matmul(ps, lhsT=st["attnT"][:, j, m * P:(m + 1) * P],
                                     rhs=st["kvch"][cc][:, j, :],
                                     start=(j == 0), stop=(j == KC - 1))
                nc.vector.tensor_add(out=yb[:, m, :], in0=ps,
                                     in1=st["img"][:, m, cs:cs + ch])
            st["kvch"][cc] = None

        def ph5_chunk(st, s, cc):
            cs = cc * ch
            hh = ch // 2
            yb = st["ybch"][cc]
            for m2 in range(MO):
                ps = psA.tile([P, ch], f32, tag="a", name="ps_out")
                for j in range(KC):
                    nc.tensor.matmul(ps, lhsT=woT_sb[:, j, m2 * P:(m2 + 1) * P],
                                     rhs=yb[:, j, :],
                                     start=(j == 0), stop=(j == KC - 1))
                # evacuation split DVE/ACT: halves the psum-bank hold time
                ot = ostage.tile([P, ch], f32, tag="ot", name="ot")
                nc.vector.tensor_copy(out=ot[:, :hh], in_=ps[:, :hh])
                nc.scalar.copy(out=ot[:, hh:], in_=ps[:, hh:])
                # pipelined-phase stores alternate the ACT HWDGE queue and
                # the gpsimd SWDGE queue (so they never head-of-line block
                # the loads on the SP queue); the final sample's stores run
                # in the drain where the SP queue is idle and fastest.
                if s < spc - 1:
                    deng = nc.scalar if m2 % 2 == 0 else nc.gpsimd
                else:
                    # drain stores split across both HWDGE queues so the
                    # ~620ns per-issue cost never paces the store stream
                    deng = nc.sync if m2 % 2 == 0 else nc.scalar
                deng.dma_start(out=out[s, m2 * P:(m2 + 1) * P, cs:cs + ch],
                               in_=ot)
            st["ybch"][cc] = None

        # ---- pipelined schedule: sample s-1's tail (last transposes, attn,
        # softmax, phases 4/5) is interleaved into sample s's pass-A chunks
        # so the PE never drains at sample boundaries.
        tails = []
        for s in range(spc):
            st = {"img": None, "kvch": [None] * NCH, "attn_ps": None,
                  "attnT": None, "imgT": [None] * NCH, "txtT": [None] * NCH,
                  "ybch": [None] * NCH}
            st["img"] = imgp.tile([P, KC, hw], f32r, tag="img", name=f"img{s}")
            for cc in range(NCH):
                passA_chunk(st, s, cc, first=(s == 0 and cc == 0))
                if s == 0 and cc < 3:
                    warmup(4)
                if cc >= 1:
                    transposes(st, s, cc - 1)
                if cc >= 2:
                    attn_chunk(st, s, cc - 2)
                # 4 pops at cc=0 gets the previous sample's softmax emitted
                # before this sample's attn chains need the psAttn banks;
                # popping only 16 of 20 leaves ~4.5us of PE work to cover
                # the final softmax's serial DVE/ACT chain
                npop = (4, 3, 3, 2, 2, 1, 1, 0)[min(cc, 7)]
                for _ in range(npop):
                    if tails:
                        tails.pop(0)()
            if s == spc - 1:
                # the final sample's softmax has no later pass-A to hide
                # behind; cover it with whatever of the previous sample's
                # tail is still pending (its last output-conv chunks).
                leftovers = tails[:]
                tails.clear()
                tails.extend([
                    (lambda st=st, s=s: transposes(st, s, NCH - 1)),
                    (lambda st=st, s=s: attn_chunk(st, s, NCH - 2)),
                    (lambda st=st, s=s: attn_chunk(st, s, NCH - 1)),
                    (lambda st=st, s=s, cov=tuple(leftovers):
                        softmax(st, s, cover=cov)),
                ])
            else:
                tails.extend([
                    (lambda st=st, s=s: transposes(st, s, NCH - 1)),
                    (lambda st=st, s=s: attn_chunk(st, s, NCH - 2)),
                    (lambda st=st, s=s: attn_chunk(st, s, NCH - 1)),
                    (lambda st=st, s=s: softmax(st, s)),
                ])
            # interleave attn@kv and output-conv chunks with a 1-chunk lag
            # so the DVE residual add of chunk c hides under the PE work of
            # chunk c+1
            tails.append(lambda st=st, s=s: ph4_chunk(st, s, 0))
            for cc in range(1, NCH):
                tails.append(lambda st=st, s=s, cc=cc: ph4_chunk(st, s, cc))
                tails.append(lambda st=st, s=s, cc=cc: ph5_chunk(st, s, cc - 1))
            tails.append(lambda st=st, s=s: ph5_chunk(st, s, NCH - 1))
        while tails:
            tails.pop(0)()

    nc.compile()
    return nc


def _get_nc():
    key = "full"
    if key not in _BUILD_CACHE:
        _BUILD_CACHE[key] = _build_nc()
    return _BUILD_CACHE[key]


LAST_RESULTS = None  # BassKernelResults of the most recent kernel() call


def kernel(x1, x2, w_img, w_txt, w_out, gamma):
    import os
    import ml_dtypes
    from concourse.bass_utils import run_bass_kernel_spmd


    x1 = np.ascontiguousarray(np.asarray(x1, dtype=np.float32)).reshape(B, C1, HW)
    x2 = np.ascontiguousarray(np.asarray(x2, dtype=np.float32)).reshape(B, C2, HW)
    w_img = np.asarray(w_img, dtype=np.float32)
    w_txt = np.asarray(w_txt, dtype=np.float32)
    w_out = np.asarray(w_out, dtype=np.float32)
    gamma = np.ascontiguousarray(np.asarray(gamma, dtype=np.float32)).reshape(1)

    w_imgT = np.ascontiguousarray(w_img.T)              # [512, 256] f32
    w_txtT = np.zeros((C2P, C), dtype=np.float32)       # [384, 256] f32, zero-pad
    w_txtT[:C2, :] = w_txt.T
    w_outT = np.ascontiguousarray(w_out.T.astype(ml_dtypes.bfloat16))  # [256, 512]

    nc = _get_nc()
    ident = np.eye(128, dtype=np.float32)
    zpad = np.zeros((64, 512), dtype=np.float32)
    in_maps = []
    for core in range(NCORES):
        s0 = core * SPC
        in_maps.append({
            "x1": np.ascontiguousarray(x1[s0:s0 + SPC]),
            "x2": np.ascontiguousarray(x2[s0:s0 + SPC]),
            "w_imgT": w_imgT,
            "w_txtT": w_txtT,
            "w_outT": w_outT,
            "gamma": gamma,
            "ident": ident,
            "zpad": zpad,
            "nonce": np.zeros((1, _nonce_len()), dtype=np.float32),
        })

    kwargs = {}
    if os.environ.get("KERNEL_TRACE"):
        kwargs["trace"] = True
        if os.environ.get("KERNEL_TRACE_DIR"):
            kwargs["tmpdir"] = os.environ["KERNEL_TRACE_DIR"]
    res = run_bass_kernel_spmd(nc, in_maps, core_ids=list(range(NCORES)), **kwargs)
    global LAST_RESULTS
    LAST_RESULTS = res
    outs = [res.results[c]["out"] for c in range(NCORES)]
    full = np.concatenate(outs, axis=0).reshape(B, C1, 64, 64)
    return full


if __name__ == "__main__":
    rng = np.random.default_rng(0)
    inputs = {
        "x1": rng.standard_normal((B, C1, 64, 64), dtype=np.float32),
        "x2": rng.standard_normal((B, C2, 64, 64), dtype=np.float32),
        "w_img": rng.standard_normal((C, C1), dtype=np.float32) / np.sqrt(C1),
        "w_txt": rng.standard_normal((C, C2), dtype=np.float32) / np.sqrt(C2),
        "w_out": rng.standard_normal((C1, C), dtype=np.float32) / np.sqrt(C),
        "gamma": rng.standard_normal(1).astype(np.float32),
    }
    out = kernel(**inputs)
    print(out.shape, out.dtype)



# revision 49
# speedup vs baseline: 1.0099x; 1.0071x over previous
"""Trainium2 Bass kernel for the CAM (cross-attention module) problem.

Math (per sample b):
    img = w_img @ x1_b          # [256, 4096]
    kv  = w_txt @ x2_b          # [256, 4096]
    attn = softmax(img @ kv^T)  # [256, 256], softmax over last dim
    y = gamma * (attn @ kv) + img
    out_b = w_out @ y           # [512, 4096]

Sharding: data-parallel over batch, 16 samples -> 2 per core x 8 cores,
no collectives.

Precision: projections / logits / attn@kv run in float32r (TRN2's
single-pass fp32 matmul mode) because the softmax is extremely
sensitive to logit error (logits ~ N(0, 64)).  The output conv runs in
bf16 (w_out and y cast to bf16): its error enters the result linearly
(~3e-3 relative), well inside the 2e-2 gate, and bf16 matmuls are much
cheaper than f32r on the PE.

Layout strategy: the spatial contraction (attn logits) needs
spatial-major operands while the residual + output conv need
channel-major ones.  img/kv are computed channel-major from the natural
HBM layout, and the spatial-major copies are made with PE transposes.
The two samples per core are software-pipelined: each sample's tail
(softmax + attn@kv + output conv) is interleaved into the next sample's
projection chunks so the tensor engine never drains.

Schedule details (roofline: ~146us of PE work/core, ~41.5 MB DMA/core):
 - weights/constants stream on the ACT HWDGE queue, x tiles on the SP
   queue (one batched rearranged 3D DMA per tensor per chunk): the
   first matmuls' operands arrive in parallel and DMA-issue slots
   (~650ns each) never pace the pipeline.
 - 8 + 12 dummy ident matmuls, interleaved into the first three chunks,
   keep the PE HAM activity window busy through the DMA-paced head so
   the clock gate opens once (1.2 -> 2.4 GHz) and stays open.
 - x2's 320 channels are NOT host-padded to 384: its three staging
   buffers are persistent tiles whose 64 dead partitions are zeroed
   once from a tiny zeros input, saving 2 MB of HBM reads per core.
 - pipelined-phase output stores alternate the ACT HWDGE and gpsimd
   SWDGE queues (never head-of-line blocking loads on the SP queue);
   the final sample's stores run in the drain, split across the SP and
   ACT queues so store issue never paces the store stream.
 - the last sample's softmax has no later projection work to hide
   behind; ~4 held-back output-conv chunks of the previous sample cover
   its serial DVE/ACT chain so the PE clock never re-throttles.
"""

import numpy as np

# Problem shapes (hardcoded per the harness contract)
B = 16
C1 = 512          # x1 channels (also output channels)
C2 = 320          # x2 channels
C2P = 384         # x2 channels padded to a multiple of 128 (K<128 matmuls
                  # and partition-offset memsets are both broken on HW)
C = 256           # projected channels
HW = 64 * 64      # spatial size
NCORES = 8
SPC = B // NCORES  # samples per core

_BUILD_CACHE = {}


def _nonce_len():
    import inspect
    import zlib
    return 2 + (zlib.crc32(inspect.getsource(_build_nc).encode()) % 997)


def _build_nc(spc=SPC, c1=C1, c2p=C2P, c=C, hw=HW, ch=512):
    """Build the per-core Bass program (same program on all cores)."""
    import concourse.tile as tile
    from concourse import bacc, mybir

    f32 = mybir.dt.float32
    f32r = mybir.dt.float32r
    bf16 = mybir.dt.bfloat16
    P = 128
    K1 = c1 // P           # k-tiles of x1 channels (4)
    K2 = c2p // P          # k-tiles of x2 channels (padded) (3)
    KC = c // P            # k-tiles of projected channels (2)
    MO = c1 // P           # m-tiles of output conv (4)
    NCH = hw // ch         # spatial chunks (8)
    TPC = ch // P          # 128-wide spatial tiles per chunk (4)
    NWARM = 10             # dummy matmuls to lift the HAM clock gate

    # Bacc (not plain Bass): its compile() runs move_matmul_waits_to_ldweights
    # + generate_event_semaphores, without which walrus rejects any Matmult
    # carrying more than one semaphore wait.
    c2 = 320               # true x2 channel count (320 = 2*128 + 64)
    C2T = c2 - 2 * P       # live rows in the last x2 k-tile (64)
    nc = bacc.Bacc("TRN2", target_bir_lowering=False)
    x1 = nc.declare_dram_parameter("x1", [spc, c1, hw], f32r, isOutput=False)
    x2 = nc.declare_dram_parameter("x2", [spc, c2, hw], f32r, isOutput=False)
    zpad = nc.declare_dram_parameter("zpad", [P - C2T, ch], f32r, isOutput=False)
    wiT = nc.declare_dram_parameter("w_imgT", [c1, c], f32r, isOutput=False)
    wtT = nc.declare_dram_parameter("w_txtT", [c2p, c], f32r, isOutput=False)
    woT = nc.declare_dram_parameter("w_outT", [c, c1], bf16, isOutput=False)
    gamma = nc.declare_dram_parameter("gamma", [1], f32, isOutput=False)
    idin = nc.declare_dram_parameter("ident", [P, P], f32r, isOutput=False)
    # The PJRT executable cache fingerprints the HLO without the embedded
    # BIR payload, so two different kernels with identical I/O signatures
    # collide. A source-hash-sized dummy input makes the signature unique.
    nc.declare_dram_parameter("nonce", [1, _nonce_len()], f32, isOutput=False)
    out = nc.declare_dram_parameter("out", [spc, c1, hw], f32, isOutput=True)

    Exp = mybir.ActivationFunctionType.Exp
    X = mybir.AxisListType.X

    with (
        tile.TileContext(nc) as tc,
        tc.tile_pool(name="singles", bufs=1) as singles,
        tc.tile_pool(name="xin", bufs=3) as xin,
        tc.tile_pool(name="tch", bufs=2) as tch,
        tc.tile_pool(name="imgp", bufs=2) as imgp,
        tc.tile_pool(name="kvp", bufs=NCH + 2) as kvp,
        tc.tile_pool(name="ybp", bufs=3) as ybp,
        tc.tile_pool(name="attnsb", bufs=2) as attnsb,
        tc.tile_pool(name="smalls", bufs=4) as smalls,
        tc.tile_pool(name="ostage", bufs=8) as ostage,
        tc.tile_pool(name="psA", bufs=4, space="PSUM") as psA,
        tc.tile_pool(name="psB", bufs=2, space="PSUM") as psB,
        tc.tile_pool(name="psAttn", bufs=2, space="PSUM") as psAttn,
    ):
        # ---- constants: ident first on the SP queue (tiny - unblocks the
        # PE warm-up immediately); the batched weight loads stream on the
        # ACT HWDGE queue in parallel with the x loads on the SP queue.
        wiT_sb = singles.tile([P, K1, c], f32r)
        wtT_sb = singles.tile([P, K2, c], f32r)
        woT_sb = singles.tile([P, KC, c1], bf16)
        ident = singles.tile([P, P], f32r)
        gamma_sb = singles.tile([P, 1], f32)

        nc.sync.dma_start(out=ident, in_=idin[:])
        # one 3D DMA per weight tensor: SBUF[p, k, c] <- W[k*128 + p, c]
        nc.scalar.dma_start(out=wiT_sb[:, :, :],
                            in_=wiT[:, :].rearrange("(k p) c -> p k c", p=P))
        nc.scalar.dma_start(out=wtT_sb[:, :, :],
                            in_=wtT[:, :].rearrange("(k p) c -> p k c", p=P))
        nc.scalar.dma_start(out=woT_sb[:, :, :],
                            in_=woT[:, :].rearrange("(k p) c -> p k c", p=P))
        nc.scalar.dma_start(out=gamma_sb, in_=gamma[:].to_broadcast((P, 1)))

        # x2 staging: 3 persistent buffers rotated manually (a tile pool
        # would treat the one-time pad zeroing below as a cross-generation
        # alias).  The dead partitions [C2T:, k=2] are zeroed once from the
        # zpad zeros input; all later DMAs only touch live regions.
        x2cs = [singles.tile([P, K2, ch], f32r, name=f"x2cbuf{i}")
                for i in range(3)]
        nc.sync.dma_start(out=x2cs[0][C2T:, K2 - 1, :], in_=zpad[:, :])
        nc.scalar.dma_start(out=x2cs[1][C2T:, K2 - 1, :], in_=zpad[:, :])
        nc.scalar.dma_start(out=x2cs[2][C2T:, K2 - 1, :], in_=zpad[:, :])

        # ---- PE clock warm-up: dummy N=128 matmuls on ident.  They only
        # depend on ident (first DMA) and write psA rotation slots nothing
        # reads.  A burst here + trickles into the first chunks (emitted in
        # passA_chunk below) keeps the HAM activity window busy through the
        # DMA-paced head so the clock gate opens once and stays open.
        def warmup(n):
            for _ in range(n):
                wps = psA.tile([P, ch], f32, tag="a", name="ps_warm")
                nc.tensor.matmul(wps[:, 0:P], lhsT=ident, rhs=ident,
                                 start=True, stop=True)

        warmup(8)

        x2c_allocs = [0]  # x2c pool allocation counter (pad-init trick)

        # ---- per-sample emission helpers -------------------------------
        def passA_chunk(st, s, cc, first=False):
            cs = cc * ch
            x1c = xin.tile([P, K1, ch], f32r, tag="x1c", name="x1c")
            x2c = x2cs[x2c_allocs[0] % 3]
            x2c_allocs[0] += 1
            if first:
                # k-granular, x1/x2 interleaved, for the first THREE chunks:
                # small DMAs complete incrementally, so the cold-start PE
                # ramp is not at the mercy of whole-MB completion latency
                # (head timing jitter was worth several us run-to-run)
                for k in range(K1):
                    nc.sync.dma_start(out=x1c[:, k, :],
                                      in_=x1[s, k * P:(k + 1) * P, cs:cs + ch])
                    if k < 2:
                        nc.sync.dma_start(out=x2c[:, k, :],
                                          in_=x2[s, k * P:(k + 1) * P, cs:cs + ch])
                    elif k == 2:
                        nc.sync.dma_start(
                            out=x2c[:C2T, 2, :],
                            in_=x2[s, 2 * P:2 * P + C2T, cs:cs + ch])
            else:
                # batched 3D DMAs (HWDGE issue slots are ~650ns each -
                # 7 per chunk would pace the whole pipeline)
                nc.sync.dma_start(
                    out=x1c[:, :, :],
                    in_=x1[s, :, cs:cs + ch].rearrange("(k p) c -> p k c", p=P))
                nc.sync.dma_start(
                    out=x2c[:, :2, :],
                    in_=x2[s, :2 * P, cs:cs + ch].rearrange("(k p) c -> p k c",
                                                            p=P))
                nc.sync.dma_start(out=x2c[:C2T, 2, :],
                                  in_=x2[s, 2 * P:2 * P + C2T, cs:cs + ch])
            for m in range(KC):
                ps = psA.tile([P, ch], f32, tag="a", name="ps_img")
                for k in range(K1):
                    nc.tensor.matmul(ps, lhsT=wiT_sb[:, k, m * P:(m + 1) * P],
                                     rhs=x1c[:, k, :],
                                     start=(k == 0), stop=(k == K1 - 1))
                nc.vector.tensor_copy(out=st["img"][:, m, cs:cs + ch], in_=ps)
            kvt = kvp.tile([P, KC, ch], f32r, tag="kv", name="kvt")
            st["kvch"][cc] = kvt
            for m in range(KC):
                ps = psA.tile([P, ch], f32, tag="a", name="ps_kv")
                for k in range(K2):
                    nc.tensor.matmul(ps, lhsT=wtT_sb[:, k, m * P:(m + 1) * P],
                                     rhs=x2c[:, k, :],
                                     start=(k == 0), stop=(k == K2 - 1))
                nc.vector.tensor_copy(out=kvt[:, m, :], in_=ps)

        def transposes(st, s, pc):
            # spatial-major orientations via PE transpose of img / kv chunks
            pcs = pc * ch
            imgT_c = tch.tile([P, TPC, c], f32r, tag="imgT", name="imgT_c")
            for t in range(TPC):
                ps = psB.tile([P, c], f32r, tag="b", name="ps_imgT")
                for i in range(KC):
                    nc.tensor.transpose(
                        ps[:, i * P:(i + 1) * P],
                        st["img"][:, i, pcs + t * P:pcs + (t + 1) * P], ident)
                nc.scalar.copy(out=imgT_c[:, t, :], in_=ps)
            txtT_c = tch.tile([P, TPC, c], f32r, tag="txtT", name="txtT_c")
            for t in range(TPC):
                ps = psB.tile([P, c], f32r, tag="b", name="ps_txtT")
                for i in range(KC):
                    nc.tensor.transpose(
                        ps[:, i * P:(i + 1) * P],
                        st["kvch"][pc][:, i, t * P:(t + 1) * P], ident)
                nc.scalar.copy(out=txtT_c[:, t, :], in_=ps)
            st["imgT"][pc] = imgT_c
            st["txtT"][pc] = txtT_c

        def attn_chunk(st, s, pc):
            if st["attn_ps"] is None:
                st["attn_ps"] = [
                    psAttn.tile([P, c], f32, tag="attn", name=f"attn{s}_{m}")
                    for m in range(KC)
                ]
            for m in range(KC):
                for t in range(TPC):
                    nc.tensor.matmul(
                        st["attn_ps"][m],
                        lhsT=st["imgT"][pc][:, t, m * P:(m + 1) * P],
                        rhs=st["txtT"][pc][:, t, :],
                        start=(pc == 0 and t == 0),
                        stop=(pc == NCH - 1 and t == TPC - 1))
            st["imgT"][pc] = st["txtT"][pc] = None

        def softmax(st, s, cover=()):
            # softmax over the free (d) axis, gamma folded in; transpose to
            # attnT [d, c] for the attn@kv contraction.  `cover` closures are
            # emitted between the DVE/ACT stats and the PE transposes so the
            # tensor engine has work while the serial softmax chain runs.
            attnT_sb = attnsb.tile([P, KC, c], f32r, tag="attnT", name="attnT")
            st["attnT"] = attnT_sb
            exps = []
            for m in range(KC):
                nmax = smalls.tile([P, 1], f32, tag="nmax", name="nmax")
                nc.vector.reduce_max(out=nmax, in_=st["attn_ps"][m], axis=X,
                                     negate=True)
                exp_sb = smalls.tile([P, c], f32r, tag="exp", name="exp_sb")
                rsum = smalls.tile([P, 1], f32, tag="rsum", name="rsum")
                nc.scalar.activation(out=exp_sb, in_=st["attn_ps"][m], func=Exp,
                                     bias=nmax, scale=1.0, accum_out=rsum)
                rg = smalls.tile([P, 1], f32, tag="rg", name="rg")
                nc.vector.reciprocal(out=rg, in_=rsum)
                nc.vector.tensor_mul(out=rg, in0=rg, in1=gamma_sb)
                nc.vector.tensor_scalar_mul(out=exp_sb, in0=exp_sb, scalar1=rg)
                exps.append(exp_sb)
            for fn in cover:
                fn()
            for m in range(KC):
                for j in range(KC):
                    pst = psB.tile([P, P], f32r, tag="b", name="ps_tr")
                    nc.tensor.transpose(pst, exps[m][:, j * P:(j + 1) * P], ident)
                    nc.vector.tensor_copy(out=attnT_sb[:, j, m * P:(m + 1) * P],
                                          in_=pst)

        def ph4_chunk(st, s, cc):
            # y = gamma*attn@kv + img, cast to bf16 for the output conv
            # (GPSIMD cannot read PSUM, so the residual add stays on DVE)
            cs = cc * ch
            yb = ybp.tile([P, KC, ch], bf16, tag="yb", name="yb")
            st["ybch"][cc] = yb
            for m in range(KC):
                ps = psA.tile([P, ch], f32, tag="a", name="ps_ai")
                for j in range(KC):
                    nc.tensor.matmul(ps, lhsT=st["attnT"][:, j, m * P:(m + 1) * P],
                                     rhs=st["kvch"][cc][:, j, :],
                                     start=(j == 0), stop=(j == KC - 1))
                nc.vector.tensor_add(out=yb[:, m, :], in0=ps,
                                     in1=st["img"][:, m, cs:cs + ch])
            st["kvch"][cc] = None

        def ph5_chunk(st, s, cc):
            cs = cc * ch
            hh = ch // 2
            yb = st["ybch"][cc]
            for m2 in range(MO):
                ps = psA.tile([P, ch], f32, tag="a", name="ps_out")
                for j in range(KC):
                    nc.tensor.matmul(ps, lhsT=woT_sb[:, j, m2 * P:(m2 + 1) * P],
                                     rhs=yb[:, j, :],
                                     start=(j == 0), stop=(j == KC - 1))
                # evacuation split DVE/ACT: halves the psum-bank hold time
                ot = ostage.tile([P, ch], f32, tag="ot", name="ot")
                nc.vector.tensor_copy(out=ot[:, :hh], in_=ps[:, :hh])
                nc.scalar.copy(out=ot[:, hh:], in_=ps[:, hh:])
                # pipelined-phase stores alternate the ACT HWDGE queue and
                # the gpsimd SWDGE queue (so they never head-of-line block
                # the loads on the SP queue); the final sample's stores run
                # in the drain where the SP queue is idle and fastest.
                if s < spc - 1:
                    deng = nc.scalar if m2 % 2 == 0 else nc.gpsimd
                else:
                    # drain stores split across both HWDGE queues so the
                    # ~620ns per-issue cost never paces the store stream
                    deng = nc.sync if m2 % 2 == 0 else nc.scalar
                deng.dma_start(out=out[s, m2 * P:(m2 + 1) * P, cs:cs + ch],
                               in_=ot)
            st["ybch"][cc] = None

        # ---- pipelined schedule: sample s-1's tail (last transposes, attn,
        # softmax, phases 4/5) is interleaved into sample s's pass-A chunks
        # so the PE never drains at sample boundaries.
        tails = []
        for s in range(spc):
            st = {"img": None, "kvch": [None] * NCH, "attn_ps": None,
                  "attnT": None, "imgT": [None] * NCH, "txtT": [None] * NCH,
                  "ybch": [None] * NCH}
            st["img"] = imgp.tile([P, KC, hw], f32r, tag="img", name=f"img{s}")
            for cc in range(NCH):
                passA_chunk(st, s, cc, first=(s == 0 and cc < 3))
                if s == 0 and cc < 3:
                    warmup(4)
                if cc >= 1:
                    transposes(st, s, cc - 1)
                if cc >= 2:
                    attn_chunk(st, s, cc - 2)
                # 4 pops at cc=0 gets the previous sample's softmax emitted
                # before this sample's attn chains need the psAttn banks;
                # popping only 16 of 20 leaves ~4.5us of PE work to cover
                # the final softmax's serial DVE/ACT chain
                npop = (4, 3, 3, 2, 2, 1, 1, 0)[min(cc, 7)]
                for _ in range(npop):
                    if tails:
                        tails.pop(0)()
            if s == spc - 1:
                # the final sample's softmax has no later pass-A to hide
                # behind; cover it with whatever of the previous sample's
                # tail is still pending (its last output-conv chunks).
                leftovers = tails[:]
                tails.clear()
                tails.extend([
                    (lambda st=st, s=s: transposes(st, s, NCH - 1)),
                    (lambda st=st, s=s: attn_chunk(st, s, NCH - 2)),
                    (lambda st=st, s=s: attn_chunk(st, s, NCH - 1)),
                    (lambda st=st, s=s, cov=tuple(leftovers):
                        softmax(st, s, cover=cov)),
                ])
            else:
                tails.extend([
                    (lambda st=st, s=s: transposes(st, s, NCH - 1)),
                    (lambda st=st, s=s: attn_chunk(st, s, NCH - 2)),
                    (lambda st=st, s=s: attn_chunk(st, s, NCH - 1)),
                    (lambda st=st, s=s: softmax(st, s)),
                ])
            # interleave attn@kv and output-conv chunks with a 1-chunk lag
            # so the DVE residual add of chunk c hides under the PE work of
            # chunk c+1
            tails.append(lambda st=st, s=s: ph4_chunk(st, s, 0))
            for cc in range(1, NCH):
                tails.append(lambda st=st, s=s, cc=cc: ph4_chunk(st, s, cc))
                tails.append(lambda st=st, s=s, cc=cc: ph5_chunk(st, s, cc - 1))
            tails.append(lambda st=st, s=s: ph5_chunk(st, s, NCH - 1))
        while tails:
            tails.pop(0)()

    nc.compile()
    return nc


def _get_nc():
    key = "full"
    if key not in _BUILD_CACHE:
        _BUILD_CACHE[key] = _build_nc()
    return _BUILD_CACHE[key]


LAST_RESULTS = None  # BassKernelResults of the most recent kernel() call


def kernel(x1, x2, w_img, w_txt, w_out, gamma):
    import os
    import ml_dtypes
    from concourse.bass_utils import run_bass_kernel_spmd


    x1 = np.ascontiguousarray(np.asarray(x1, dtype=np.float32)).reshape(B, C1, HW)
    x2 = np.ascontiguousarray(np.asarray(x2, dtype=np.float32)).reshape(B, C2, HW)
    w_img = np.asarray(w_img, dtype=np.float32)
    w_txt = np.asarray(w_txt, dtype=np.float32)
    w_out = np.asarray(w_out, dtype=np.float32)
    gamma = np.ascontiguousarray(np.asarray(gamma, dtype=np.float32)).reshape(1)

    w_imgT = np.ascontiguousarray(w_img.T)              # [512, 256] f32
    w_txtT = np.zeros((C2P, C), dtype=np.float32)       # [384, 256] f32, zero-pad
    w_txtT[:C2, :] = w_txt.T
    w_outT = np.ascontiguousarray(w_out.T.astype(ml_dtypes.bfloat16))  # [256, 512]

    nc = _get_nc()
    ident = np.eye(128, dtype=np.float32)
    zpad = np.zeros((64, 512), dtype=np.float32)
    in_maps = []
    for core in range(NCORES):
        s0 = core * SPC
        in_maps.append({
            "x1": np.ascontiguousarray(x1[s0:s0 + SPC]),
            "x2": np.ascontiguousarray(x2[s0:s0 + SPC]),
            "w_imgT": w_imgT,
            "w_txtT": w_txtT,
            "w_outT": w_outT,
            "gamma": gamma,
            "ident": ident,
            "zpad": zpad,
            "nonce": np.zeros((1, _nonce_len()), dtype=np.float32),
        })

    kwargs = {}
    if os.environ.get("KERNEL_TRACE"):
        kwargs["trace"] = True
        if os.environ.get("KERNEL_TRACE_DIR"):
            kwargs["tmpdir"] = os.environ["KERNEL_TRACE_DIR"]
    res = run_bass_kernel_spmd(nc, in_maps, core_ids=list(range(NCORES)), **kwargs)
    global LAST_RESULTS
    LAST_RESULTS = res
    outs = [res.results[c]["out"] for c in range(NCORES)]
    full = np.concatenate(outs, axis=0).reshape(B, C1, 64, 64)
    return full


if __name__ == "__main__":
    rng = np.random.default_rng(0)
    inputs = {
        "x1": rng.standard_normal((B, C1, 64, 64), dtype=np.float32),
        "x2": rng.standard_normal((B, C2, 64, 64), dtype=np.float32),
        "w_img": rng.standard_normal((C, C1), dtype=np.float32) / np.sqrt(C1),
        "w_txt": rng.standard_normal((C, C2), dtype=np.float32) / np.sqrt(C2),
        "w_out": rng.standard_normal((C1, C), dtype=np.float32) / np.sqrt(C),
        "gamma": rng.standard_normal(1).astype(np.float32),
    }
    out = kernel(**inputs)
    print(out.shape, out.dtype)

